# revision 23
# baseline (speedup 1.0000x reference)
"""Trainium2 Bass kernel for CorrelationModule (per-pixel self-attention).

Math (per batch element b, all fp32):
  xf = x[b] reshaped [C=384, N=2304]
  q = Wq@xf + bq, k = Wk@xf + bk, v = Wv@xf + bv       (1x1 convs)
  attn = softmax_m(q^T k / sqrt(512))                  (N x N)
  out = Wo @ (v @ attn^T) + bo                         -> [512, N]

Sharding: batch B=8 data-parallel across the 8 NeuronCores, params replicated.

Per-core kernel layout choices:
  - Scores are computed TRANSPOSED: s_t[m, n] = sum_o k[o,m] q[o,n], so the
    softmax reduction (over m) lands on the PSUM partition axis and is done
    with a ones-vector matmul on the TensorEngine (no 128x128 transposes).
  - exp is taken without max-subtraction: scores*scale ~ N(0, 1/9), so
    exp() cannot overflow for this module's data distribution.
  - Softmax normalization is deferred: AV and the Wo projection run on the
    unnormalized exp-scores; the final tile is multiplied by the broadcast
    reciprocal row sums.  bv is folded into bo' = Wo@bv + bo on the host
    (valid because sum_m attn = 1 after normalization).
  - Matmul operands are fp16 (1 row/cycle PE rate + fast weight load;
    fp32r was full-rate too but its 4-byte LDWEIGHTS at ~189 ns/MM was
    ~25% of the kernel).  PSUM accumulation stays fp32.
  - The two big attention matmuls (scores ~2.7 GMAC and AV ~2.7 GMAC of
    the 7.4 GMAC total) run in fp8e4 with perf_mode=DoubleRow: operands
    are laid out [128, 2, free] so each matmul contracts 256 (2 fp8 per
    PE cell, ~2 MAC/cell/cycle) — half the matmul count at ~1.4x the
    per-matmul rate.  Projections stay fp16 for accuracy; softmax
    normalization uses the same quantized e for numerator+denominator,
    so fp8 e-rounding cancels to first order.
"""

import numpy as np

B, C, O, H, W = 8, 384, 512, 48, 48
N = H * W  # 2304 tokens
P = 128
CT, OT, MT = C // P, O // P, N // P  # 3, 4, 18
NBLK = [(0, 512), (512, 512), (1024, 512), (1536, 512), (2048, 256)]
SCALE = 1.0 / float(np.sqrt(O))

_cache = {}


def _build_nc():
    import concourse.bacc as bacc
    import concourse.tile as tile
    import concourse.mybir as mybir

    F32 = mybir.dt.float32
    F16 = mybir.dt.float16
    F8 = mybir.dt.float8e4
    DR = mybir.MatmulPerfMode.DoubleRow

    nc = bacc.Bacc(
        "TRN2",
        target_bir_lowering=False,
        debug=False,
        enable_asserts=False,
        num_devices=1,
    )

    xf_d = nc.dram_tensor("xf", [C, N], F16, kind="ExternalInput").ap()
    wqkv_d = nc.dram_tensor("wqkv", [C, 3 * O], F16, kind="ExternalInput").ap()
    wot_d = nc.dram_tensor("wot", [O, O], F16, kind="ExternalInput").ap()
    bias_d = nc.dram_tensor("bias", [O, 3], F32, kind="ExternalInput").ap()
    ones_d = nc.dram_tensor("ones", [P, P], F16, kind="ExternalInput").ap()
    y_d = nc.dram_tensor("y", [O, N], F32, kind="ExternalOutput").ap()

    with tile.TileContext(nc) as tc:
        with (
            nc.allow_low_precision(reason="fp16 matmul operands"),
            tc.tile_pool(name="const", bufs=1) as const,
            tc.tile_pool(name="work", bufs=1) as work,
            tc.tile_pool(name="ps", bufs=1, space="PSUM") as ps,
        ):
            # ---- persistent SBUF tensors -------------------------------
            xf_sb = [
                const.tile([P, N], F16, tag=f"xf{c}", name=f"xf_sb{c}")
                for c in range(CT)
            ]
            wqkv_sb = [
                const.tile([P, 3 * O], F16, tag=f"wqkv{c}", name=f"wqkv_sb{c}")
                for c in range(CT)
            ]
            wqt_sb = [t[:, 0:O] for t in wqkv_sb]
            wkt_sb = [t[:, O:2 * O] for t in wqkv_sb]
            wvt_sb = [t[:, 2 * O:3 * O] for t in wqkv_sb]
            wot_sb = [
                const.tile([P, O], F16, tag=f"wot{o}", name=f"wot_sb{o}")
                for o in range(OT)
            ]
            bias_sb = [
                const.tile([P, 3], F32, tag=f"bias{o}", name=f"bias_sb{o}")
                for o in range(OT)
            ]
            bq_sb = [t[:, 0:1] for t in bias_sb]
            bk_sb = [t[:, 1:2] for t in bias_sb]
            bo2_sb = [t[:, 2:3] for t in bias_sb]
            # fp8 DoubleRow operand layouts: [128, 2, free] where dim1
            # selects the paired contraction element (o = 256c + 128j + p
            # for K/Q halves c; m-tile = 2*m2 + j for V/E pairs m2).
            kc_sb = [
                const.tile([P, 2, N], F8, tag=f"kc{c}", name=f"kc_sb{c}")
                for c in range(2)
            ]
            vtc_sb = [
                const.tile([P, 2, O], F8, tag=f"vtc{m2}", name=f"vtc_sb{m2}")
                for m2 in range(MT // 2)
            ]
            ones_sb = const.tile([P, P], F16, tag="ones", name="ones_sb")
            # load order tuned for time-to-first-matmul and phase-1
            # just-in-time arrival: Wk + first xf n-block first, the rest
            # of xf in n-block chunks split across the sync/gpsimd queues,
            # weights for later phases behind them on the scalar queue.
            for c in range(CT):
                nc.scalar.dma_start(wqkv_sb[c][:, O:2 * O],
                                    wqkv_d[c * P:(c + 1) * P, O:2 * O])
                nc.sync.dma_start(xf_sb[c][:, 0:512],
                                  xf_d[c * P:(c + 1) * P, 0:512])
            for o in range(OT):
                nc.gpsimd.dma_start(bias_sb[o][:], bias_d[o * P:(o + 1) * P, :])
            nc.gpsimd.dma_start(ones_sb[:], ones_d[:, :])
            for c in range(CT):
                nc.sync.dma_start(xf_sb[c][:, 512:1024],
                                  xf_d[c * P:(c + 1) * P, 512:1024])
            for c in range(CT):
                nc.scalar.dma_start(wqkv_sb[c][:, 2 * O:3 * O],
                                    wqkv_d[c * P:(c + 1) * P, 2 * O:3 * O])
            for c in range(CT):
                nc.sync.dma_start(xf_sb[c][:, 1024:1536],
                                  xf_d[c * P:(c + 1) * P, 1024:1536])
                nc.gpsimd.dma_start(xf_sb[c][:, 1536:2304],
                                    xf_d[c * P:(c + 1) * P, 1536:2304])
            for c in range(CT):
                nc.scalar.dma_start(wqkv_sb[c][:, 0:O],
                                    wqkv_d[c * P:(c + 1) * P, 0:O])
            for o in range(OT):
                nc.scalar.dma_start(wot_sb[o][:], wot_d[o * P:(o + 1) * P, :])

            # ---- phase 1: K = Wk@xf + bk (layout [o, m]) interleaved
            # with V^T = (Wv@xf)^T (layout [m, o]), ordered so each step
            # only consumes xf columns already landed by the chunked DMAs
            def k_block(n0, nw):
                for o in range(OT):
                    osl = slice(o * P, (o + 1) * P)
                    kp = ps.tile([P, nw], F32, tag="s", bufs=4, name=f"kp_{o}_{n0}")
                    for c in range(CT):
                        nc.tensor.matmul(
                            kp[:],
                            wkt_sb[c][:, osl],
                            xf_sb[c][:, n0:n0 + nw],
                            start=(c == 0),
                            stop=(c == CT - 1),
                        )
                    nc.scalar.add(kc_sb[o // 2][:, o % 2, n0:n0 + nw],
                                  kp[:], bk_sb[o][:])

            def v_tiles(ms):
                for m in ms:
                    msl = slice(m * P, (m + 1) * P)
                    vp = ps.tile([P, O], F32, tag="s", bufs=4, name=f"vp_{m}")
                    for c in range(CT):
                        nc.tensor.matmul(
                            vp[:],
                            xf_sb[c][:, msl],
                            wvt_sb[c][:],
                            start=(c == 0),
                            stop=(c == CT - 1),
                        )
                    nc.vector.tensor_copy(vtc_sb[m // 2][:, m % 2, :], vp[:])

            k_block(*NBLK[0])
            k_block(*NBLK[1])
            v_tiles(range(0, 8))
            k_block(*NBLK[2])
            v_tiles(range(8, 12))
            k_block(*NBLK[3])
            v_tiles(range(12, 16))
            k_block(*NBLK[4])
            v_tiles(range(16, 18))

            # ---- phase 2: flash attention over n-blocks ----------------
            # The per-block finish (Wo projection + normalize + store) is
            # deferred until after the NEXT block's Q projection, so the PE
            # never waits on the av->SBUF copies at a block boundary.
            pending_finish = None
            for n0, nw in NBLK:
                nsl = slice(n0, n0 + nw)
                # Q for this block (fp8 DoubleRow layout [p, j, n]), bias
                # bq added during the PSUM->SBUF downcast
                qc_sb = [
                    work.tile([P, 2, nw], F8, tag=f"qc{c}", bufs=3,
                              name=f"qc_{n0}_{c}")
                    for c in range(2)
                ]
                for o in range(OT):
                    osl = slice(o * P, (o + 1) * P)
                    qp = ps.tile([P, nw], F32, tag="s", bufs=4, name=f"qp_{n0}_{o}")
                    for c in range(CT):
                        nc.tensor.matmul(
                            qp[:],
                            wqt_sb[c][:, osl],
                            xf_sb[c][:, nsl],
                            start=(c == 0),
                            stop=(c == CT - 1),
                        )
                    nc.scalar.add(qc_sb[o // 2][:, o % 2, :], qp[:],
                                  bq_sb[o][:])

                av_ps = [
                    ps.tile([P, nw], F32, tag=f"av{o}", bufs=1,
                            name=f"av_{n0}_{o}")
                    for o in range(OT)
                ]
                # fp16 accumulation: 2x DVE rate, and directly usable as
                # the ones-matmul moving operand for the column sums
                eacc = work.tile([P, nw], F16, tag="eacc", bufs=2,
                                 name=f"eacc_{n0}")
                # paired m-tiles: 4 DoubleRow score matmuls then 4 DoubleRow
                # AV matmuls per m2 (each contracting 256)
                for m2 in range(MT // 2):
                    # previous block's epilogue lands here, two iterations
                    # in, so its PSUM reads/DVE work never stall this
                    # block's pipeline warm-up
                    if m2 == 2 and pending_finish is not None:
                        pending_finish()
                        pending_finish = None
                    ec = work.tile([P, 2, nw], F8, tag="e", bufs=3,
                                   name=f"e_{n0}_{m2}")
                    for jj in (0, 1):
                        m = 2 * m2 + jj
                        msl = slice(m * P, (m + 1) * P)
                        sp = ps.tile([P, nw], F32, tag="s", bufs=4,
                                     name=f"sp_{n0}_{m}")
                        for c in range(2):
                            nc.tensor.matmul(
                                sp[:],
                                kc_sb[c][:, :, msl],
                                qc_sb[c][:, :, :],
                                start=(c == 0),
                                stop=(c == 1),
                                perf_mode=DR,
                            )
                        nc.scalar.activation(
                            ec[:, jj, :], sp[:],
                            mybir.ActivationFunctionType.Exp,
                            scale=SCALE,
                        )
                        if m == 0:
                            nc.vector.tensor_copy(eacc[:], ec[:, jj, :])
                        else:
                            nc.vector.tensor_add(eacc[:], eacc[:],
                                                 ec[:, jj, :])
                    for o in range(OT):
                        osl = slice(o * P, (o + 1) * P)
                        nc.tensor.matmul(
                            av_ps[o][:],
                            vtc_sb[m2][:, :, osl],
                            ec[:, :, :],
                            start=(m2 == 0),
                            stop=(m2 == MT // 2 - 1),
                            perf_mode=DR,
                        )

                # av -> SBUF first so these DVE/ACT copies are not queued
                # behind the reciprocal (which blocks on the all-reduce)
                av_sb = []
                for o in range(OT):
                    t = work.tile([P, nw], F16, tag=f"av_sb{o}", bufs=2,
                                  name=f"avs_{n0}_{o}")
                    if o % 2 == 0:
                        nc.vector.tensor_copy(t[:], av_ps[o][:])
                    else:
                        nc.scalar.copy(t[:], av_ps[o][:])
                    av_sb.append(t)

                def make_finish(n0=n0, nw=nw, nsl=nsl, av_sb=av_sb,
                                eacc=eacc):
                    def finish():
                        # denominator: ones-matmul column-sums the fp16
                        # exp accumulator over partitions — every output
                        # partition gets the column sums (broadcast for
                        # free), in one ~0.4us TensorE op instead of a
                        # ~3.5us gpsimd partition_all_reduce that stalled
                        # the PSUM rotation at every block boundary.
                        dsum = ps.tile([P, nw], F32, tag="s", bufs=4,
                                       name=f"dsum_{n0}")
                        nc.tensor.matmul(dsum[:], ones_sb[:], eacc[:],
                                         start=True, stop=True)
                        rb = work.tile([P, nw], F32, tag="rb_sb", bufs=2,
                                       name=f"rb_{n0}")
                        nc.vector.reciprocal_approx_fast(out=rb[:], in_=dsum[:])
                        for p in range(4):
                            psl = slice(p * P, (p + 1) * P)
                            pp = ps.tile([P, nw], F32, tag="s", bufs=4,
                                         name=f"pp_{n0}_{p}")
                            for o in range(OT):
                                nc.tensor.matmul(
                                    pp[:],
                                    wot_sb[o][:, psl],
                                    av_sb[o][:],
                                    start=(o == 0),
                                    stop=(o == OT - 1),
                                )
                            tmp = work.tile([P, nw], F32, tag="tmp", bufs=2,
                                            name=f"tmp_{n0}_{p}")
                            nc.vector.tensor_mul(tmp[:], pp[:], rb[:])
                            # bias add on gpsimd (SBUF-only op): keeps the
                            # ACT queue clear for exps/bias-adds at block
                            # boundaries
                            outt = work.tile([P, nw], F32, tag="out", bufs=2,
                                             name=f"out_{n0}_{p}")
                            nc.gpsimd.tensor_scalar_add(outt[:], tmp[:],
                                                        bo2_sb[p][:])
                            if p % 2 == 0:
                                nc.sync.dma_start(y_d[psl, nsl], outt[:])
                            else:
                                nc.gpsimd.dma_start(y_d[psl, nsl], outt[:])
                    return finish

                pending_finish = make_finish()

            pending_finish()

    nc.compile()
    return nc


def get_nc():
    if "nc" not in _cache:
        _cache["nc"] = _build_nc()
    return _cache["nc"]


def make_in_maps(x, Wq, bq, Wk, bk, Wv, bv, Wo, bo):
    x = np.asarray(x, np.float32)
    Wq = np.asarray(Wq, np.float32)
    Wk = np.asarray(Wk, np.float32)
    Wv = np.asarray(Wv, np.float32)
    Wo = np.asarray(Wo, np.float32)
    bq = np.asarray(bq, np.float32)
    bk = np.asarray(bk, np.float32)
    bv = np.asarray(bv, np.float32)
    bo = np.asarray(bo, np.float32)

    wqkv = np.concatenate([Wq.T, Wk.T, Wv.T], axis=1).astype(np.float16)
    wot = np.ascontiguousarray(Wo.T).astype(np.float16)
    bo2 = (Wo @ bv + bo).astype(np.float32)
    bias = np.stack([bq, bk, bo2], axis=1).astype(np.float32)

    xf = x.reshape(B, C, N).astype(np.float16)
    shared = {
        "wqkv": np.ascontiguousarray(wqkv),
        "wot": wot,
        "bias": np.ascontiguousarray(bias),
        "ones": np.ones((128, 128), np.float16),
    }
    return [
        {"xf": np.ascontiguousarray(xf[b]), **shared} for b in range(B)
    ]


def kernel(x, Wq, bq, Wk, bk, Wv, bv, Wo, bo):
    from concourse import bass_utils

    nc = get_nc()
    in_maps = make_in_maps(x, Wq, bq, Wk, bk, Wv, bv, Wo, bo)
    res = bass_utils.run_bass_kernel_spmd(nc, in_maps, core_ids=list(range(B)))
    y = np.stack([res.results[b]["y"] for b in range(B)], axis=0)
    return np.ascontiguousarray(y.reshape(B, O, H, W))



# revision 24
# speedup vs baseline: 1.6605x; 1.6605x over previous
"""Trainium2 Bass kernel for CorrelationModule (per-pixel self-attention).

Math (per batch element b, all fp32):
  xf = x[b] reshaped [C=384, N=2304]
  q = Wq@xf + bq, k = Wk@xf + bk, v = Wv@xf + bv       (1x1 convs)
  attn = softmax_m(q^T k / sqrt(512))                  (N x N)
  out = Wo @ (v @ attn^T) + bo                         -> [512, N]

Sharding: batch B=8 data-parallel across the 8 NeuronCores, params replicated.

Per-core kernel layout choices:
  - Scores are computed TRANSPOSED: s_t[m, n] = sum_o k[o,m] q[o,n], so the
    softmax reduction (over m) lands on the PSUM partition axis and is done
    with a ones-vector matmul on the TensorEngine (no 128x128 transposes).
  - exp is taken without max-subtraction: scores*scale ~ N(0, 1/9), so
    exp() cannot overflow for this module's data distribution.
  - Softmax normalization is deferred: AV and the Wo projection run on the
    unnormalized exp-scores; the final tile is multiplied by the broadcast
    reciprocal row sums.  bv is folded into bo' = Wo@bv + bo on the host
    (valid because sum_m attn = 1 after normalization).
  - Matmul operands are fp16 (1 row/cycle PE rate + fast weight load;
    fp32r was full-rate too but its 4-byte LDWEIGHTS at ~189 ns/MM was
    ~25% of the kernel).  PSUM accumulation stays fp32.
  - The two big attention matmuls (scores ~2.7 GMAC and AV ~2.7 GMAC of
    the 7.4 GMAC total) run in fp8e4 with perf_mode=DoubleRow: operands
    are laid out [128, 2, free] so each matmul contracts 256 (2 fp8 per
    PE cell, ~2 MAC/cell/cycle) — half the matmul count at ~1.4x the
    per-matmul rate.  Projections stay fp16 for accuracy; softmax
    normalization uses the same quantized e for numerator+denominator,
    so fp8 e-rounding cancels to first order.
"""

import numpy as np

B, C, O, H, W = 8, 384, 512, 48, 48
N = H * W  # 2304 tokens
P = 128
CT, OT, MT = C // P, O // P, N // P  # 3, 4, 18
NBLK = [(0, 512), (512, 512), (1024, 512), (1536, 512), (2048, 256)]
SCALE = 1.0 / float(np.sqrt(O))

_cache = {}


def _build_nc():
    import concourse.bacc as bacc
    import concourse.tile as tile
    import concourse.mybir as mybir

    F32 = mybir.dt.float32
    F16 = mybir.dt.float16
    F8 = mybir.dt.float8e4
    DR = mybir.MatmulPerfMode.DoubleRow

    nc = bacc.Bacc(
        "TRN2",
        target_bir_lowering=False,
        debug=False,
        enable_asserts=False,
        num_devices=1,
    )

    xf_d = nc.dram_tensor("xf", [C, N], F16, kind="ExternalInput").ap()
    wqkv_d = nc.dram_tensor("wqkv", [C, 3 * O], F16, kind="ExternalInput").ap()
    wot_d = nc.dram_tensor("wot", [O, O], F16, kind="ExternalInput").ap()
    bias_d = nc.dram_tensor("bias", [O, 3], F32, kind="ExternalInput").ap()
    ones_d = nc.dram_tensor("ones", [P, P], F16, kind="ExternalInput").ap()
    y_d = nc.dram_tensor("y", [O, N], F32, kind="ExternalOutput").ap()

    with tile.TileContext(nc) as tc:
        with (
            nc.allow_low_precision(reason="fp16 matmul operands"),
            tc.tile_pool(name="const", bufs=1) as const,
            tc.tile_pool(name="work", bufs=1) as work,
            tc.tile_pool(name="ps", bufs=1, space="PSUM") as ps,
        ):
            # ---- persistent SBUF tensors -------------------------------
            xf_sb = [
                const.tile([P, N], F16, tag=f"xf{c}", name=f"xf_sb{c}")
                for c in range(CT)
            ]
            wqkv_sb = [
                const.tile([P, 3 * O], F16, tag=f"wqkv{c}", name=f"wqkv_sb{c}")
                for c in range(CT)
            ]
            wqt_sb = [t[:, 0:O] for t in wqkv_sb]
            wkt_sb = [t[:, O:2 * O] for t in wqkv_sb]
            wvt_sb = [t[:, 2 * O:3 * O] for t in wqkv_sb]
            wot_sb = [
                const.tile([P, O], F16, tag=f"wot{o}", name=f"wot_sb{o}")
                for o in range(OT)
            ]
            bias_sb = [
                const.tile([P, 3], F32, tag=f"bias{o}", name=f"bias_sb{o}")
                for o in range(OT)
            ]
            bq_sb = [t[:, 0:1] for t in bias_sb]
            bk_sb = [t[:, 1:2] for t in bias_sb]
            bo2_sb = [t[:, 2:3] for t in bias_sb]
            # fp8 DoubleRow operand layouts: [128, 2, free] where dim1
            # selects the paired contraction element (o = 256c + 128j + p
            # for K/Q halves c; m-tile = 2*m2 + j for V/E pairs m2).
            kc_sb = [
                const.tile([P, 2, N], F8, tag=f"kc{c}", name=f"kc_sb{c}")
                for c in range(2)
            ]
            vtc_sb = [
                const.tile([P, 2, O], F8, tag=f"vtc{m2}", name=f"vtc_sb{m2}")
                for m2 in range(MT // 2)
            ]
            ones_sb = const.tile([P, P], F16, tag="ones", name="ones_sb")
            # load order tuned for time-to-first-matmul and phase-1
            # just-in-time arrival: Wk + first xf n-block first, the rest
            # of xf in n-block chunks split across the sync/gpsimd queues,
            # weights for later phases behind them on the scalar queue.
            for c in range(CT):
                nc.scalar.dma_start(wqkv_sb[c][:, O:2 * O],
                                    wqkv_d[c * P:(c + 1) * P, O:2 * O])
                nc.sync.dma_start(xf_sb[c][:, 0:512],
                                  xf_d[c * P:(c + 1) * P, 0:512])
            for o in range(OT):
                nc.gpsimd.dma_start(bias_sb[o][:], bias_d[o * P:(o + 1) * P, :])
            nc.gpsimd.dma_start(ones_sb[:], ones_d[:, :])
            for c in range(CT):
                nc.sync.dma_start(xf_sb[c][:, 512:1024],
                                  xf_d[c * P:(c + 1) * P, 512:1024])
            for c in range(CT):
                nc.scalar.dma_start(wqkv_sb[c][:, 2 * O:3 * O],
                                    wqkv_d[c * P:(c + 1) * P, 2 * O:3 * O])
            for c in range(CT):
                nc.sync.dma_start(xf_sb[c][:, 1024:1536],
                                  xf_d[c * P:(c + 1) * P, 1024:1536])
                nc.gpsimd.dma_start(xf_sb[c][:, 1536:2304],
                                    xf_d[c * P:(c + 1) * P, 1536:2304])
            for c in range(CT):
                nc.scalar.dma_start(wqkv_sb[c][:, 0:O],
                                    wqkv_d[c * P:(c + 1) * P, 0:O])
            for o in range(OT):
                nc.scalar.dma_start(wot_sb[o][:], wot_d[o * P:(o + 1) * P, :])

            # ---- phase 1: K = Wk@xf + bk (layout [o, m]) interleaved
            # with V^T = (Wv@xf)^T (layout [m, o]), ordered so each step
            # only consumes xf columns already landed by the chunked DMAs
            def k_block(n0, nw):
                for o in range(OT):
                    osl = slice(o * P, (o + 1) * P)
                    kp = ps.tile([P, nw], F32, tag="s", bufs=4, name=f"kp_{o}_{n0}")
                    for c in range(CT):
                        nc.tensor.matmul(
                            kp[:],
                            wkt_sb[c][:, osl],
                            xf_sb[c][:, n0:n0 + nw],
                            start=(c == 0),
                            stop=(c == CT - 1),
                        )
                    nc.scalar.add(kc_sb[o // 2][:, o % 2, n0:n0 + nw],
                                  kp[:], bk_sb[o][:])

            def v_tiles(ms):
                for m in ms:
                    msl = slice(m * P, (m + 1) * P)
                    vp = ps.tile([P, O], F32, tag="s", bufs=4, name=f"vp_{m}")
                    for c in range(CT):
                        nc.tensor.matmul(
                            vp[:],
                            xf_sb[c][:, msl],
                            wvt_sb[c][:],
                            start=(c == 0),
                            stop=(c == CT - 1),
                        )
                    nc.vector.tensor_copy(vtc_sb[m // 2][:, m % 2, :], vp[:])

            k_block(*NBLK[0])
            k_block(*NBLK[1])
            v_tiles(range(0, 8))
            k_block(*NBLK[2])
            v_tiles(range(8, 12))
            k_block(*NBLK[3])
            v_tiles(range(12, 16))
            k_block(*NBLK[4])
            v_tiles(range(16, 18))

            # ---- phase 2: flash attention over n-blocks ----------------
            # The per-block finish (Wo projection + normalize + store) is
            # deferred until after the NEXT block's Q projection, so the PE
            # never waits on the av->SBUF copies at a block boundary.
            pending_finish = None
            for n0, nw in NBLK:
                nsl = slice(n0, n0 + nw)
                # Q for this block (fp8 DoubleRow layout [p, j, n]), bias
                # bq added during the PSUM->SBUF downcast
                qc_sb = [
                    work.tile([P, 2, nw], F8, tag=f"qc{c}", bufs=3,
                              name=f"qc_{n0}_{c}")
                    for c in range(2)
                ]
                for o in range(OT):
                    osl = slice(o * P, (o + 1) * P)
                    qp = ps.tile([P, nw], F32, tag="s", bufs=4, name=f"qp_{n0}_{o}")
                    for c in range(CT):
                        nc.tensor.matmul(
                            qp[:],
                            wqt_sb[c][:, osl],
                            xf_sb[c][:, nsl],
                            start=(c == 0),
                            stop=(c == CT - 1),
                        )
                    nc.scalar.add(qc_sb[o // 2][:, o % 2, :], qp[:],
                                  bq_sb[o][:])

                av_ps = [
                    ps.tile([P, nw], F32, tag=f"av{o}", bufs=1,
                            name=f"av_{n0}_{o}")
                    for o in range(OT)
                ]
                # fp16 accumulation: 2x DVE rate, and directly usable as
                # the ones-matmul moving operand for the column sums
                eacc = work.tile([P, nw], F16, tag="eacc", bufs=2,
                                 name=f"eacc_{n0}")
                # paired m-tiles: 4 DoubleRow score matmuls then 4 DoubleRow
                # AV matmuls per m2 (each contracting 256)
                for m2 in range(MT // 2):
                    # previous block's epilogue lands here, two iterations
                    # in, so its PSUM reads/DVE work never stall this
                    # block's pipeline warm-up
                    if m2 == 2 and pending_finish is not None:
                        pending_finish()
                        pending_finish = None
                    ec = work.tile([P, 2, nw], F8, tag="e", bufs=3,
                                   name=f"e_{n0}_{m2}")
                    for jj in (0, 1):
                        m = 2 * m2 + jj
                        msl = slice(m * P, (m + 1) * P)
                        sp = ps.tile([P, nw], F32, tag="s", bufs=4,
                                     name=f"sp_{n0}_{m}")
                        for c in range(2):
                            nc.tensor.matmul(
                                sp[:],
                                kc_sb[c][:, :, msl],
                                qc_sb[c][:, :, :],
                                start=(c == 0),
                                stop=(c == 1),
                                perf_mode=DR,
                            )
                        nc.scalar.activation(
                            ec[:, jj, :], sp[:],
                            mybir.ActivationFunctionType.Exp,
                            scale=SCALE,
                        )
                        if m == 0:
                            nc.vector.tensor_copy(eacc[:], ec[:, jj, :])
                        else:
                            nc.vector.tensor_add(eacc[:], eacc[:],
                                                 ec[:, jj, :])
                    for o in range(OT):
                        osl = slice(o * P, (o + 1) * P)
                        nc.tensor.matmul(
                            av_ps[o][:],
                            vtc_sb[m2][:, :, osl],
                            ec[:, :, :],
                            start=(m2 == 0),
                            stop=(m2 == MT // 2 - 1),
                            perf_mode=DR,
                        )

                # av -> SBUF first so these DVE/ACT copies are not queued
                # behind the reciprocal (which blocks on the all-reduce)
                av_sb = []
                for o in range(OT):
                    t = work.tile([P, nw], F16, tag=f"av_sb{o}", bufs=2,
                                  name=f"avs_{n0}_{o}")
                    if o % 2 == 0:
                        nc.vector.tensor_copy(t[:], av_ps[o][:])
                    else:
                        nc.scalar.copy(t[:], av_ps[o][:])
                    av_sb.append(t)

                def make_finish(n0=n0, nw=nw, nsl=nsl, av_sb=av_sb,
                                eacc=eacc):
                    def finish():
                        # denominator: ones-matmul column-sums the fp16
                        # exp accumulator over partitions — every output
                        # partition gets the column sums (broadcast for
                        # free), in one ~0.4us TensorE op instead of a
                        # ~3.5us gpsimd partition_all_reduce that stalled
                        # the PSUM rotation at every block boundary.
                        dsum = ps.tile([P, nw], F32, tag="s", bufs=4,
                                       name=f"dsum_{n0}")
                        nc.tensor.matmul(dsum[:], ones_sb[:], eacc[:],
                                         start=True, stop=True)
                        rb = work.tile([P, nw], F32, tag="rb_sb", bufs=2,
                                       name=f"rb_{n0}")
                        nc.vector.reciprocal_approx_fast(out=rb[:], in_=dsum[:])
                        for p in range(4):
                            psl = slice(p * P, (p + 1) * P)
                            pp = ps.tile([P, nw], F32, tag="s", bufs=4,
                                         name=f"pp_{n0}_{p}")
                            for o in range(OT):
                                nc.tensor.matmul(
                                    pp[:],
                                    wot_sb[o][:, psl],
                                    av_sb[o][:],
                                    start=(o == 0),
                                    stop=(o == OT - 1),
                                )
                            tmp = work.tile([P, nw], F32, tag="tmp", bufs=2,
                                            name=f"tmp_{n0}_{p}")
                            nc.vector.tensor_mul(tmp[:], pp[:], rb[:])
                            outt = work.tile([P, nw], F32, tag="out", bufs=2,
                                             name=f"out_{n0}_{p}")
                            nc.scalar.add(outt[:], tmp[:], bo2_sb[p][:])
                            nc.sync.dma_start(y_d[psl, nsl], outt[:])
                    return finish

                pending_finish = make_finish()

            pending_finish()

    nc.compile()
    return nc


def get_nc():
    if "nc" not in _cache:
        _cache["nc"] = _build_nc()
    return _cache["nc"]


def make_in_maps(x, Wq, bq, Wk, bk, Wv, bv, Wo, bo):
    x = np.asarray(x, np.float32)
    Wq = np.asarray(Wq, np.float32)
    Wk = np.asarray(Wk, np.float32)
    Wv = np.asarray(Wv, np.float32)
    Wo = np.asarray(Wo, np.float32)
    bq = np.asarray(bq, np.float32)
    bk = np.asarray(bk, np.float32)
    bv = np.asarray(bv, np.float32)
    bo = np.asarray(bo, np.float32)

    wqkv = np.concatenate([Wq.T, Wk.T, Wv.T], axis=1).astype(np.float16)
    wot = np.ascontiguousarray(Wo.T).astype(np.float16)
    bo2 = (Wo @ bv + bo).astype(np.float32)
    bias = np.stack([bq, bk, bo2], axis=1).astype(np.float32)

    xf = x.reshape(B, C, N).astype(np.float16)
    shared = {
        "wqkv": np.ascontiguousarray(wqkv),
        "wot": wot,
        "bias": np.ascontiguousarray(bias),
        "ones": np.ones((128, 128), np.float16),
    }
    return [
        {"xf": np.ascontiguousarray(xf[b]), **shared} for b in range(B)
    ]


def kernel(x, Wq, bq, Wk, bk, Wv, bv, Wo, bo):
    from concourse import bass_utils

    nc = get_nc()
    in_maps = make_in_maps(x, Wq, bq, Wk, bk, Wv, bv, Wo, bo)
    res = bass_utils.run_bass_kernel_spmd(nc, in_maps, core_ids=list(range(B)))
    y = np.stack([res.results[b]["y"] for b in range(B)], axis=0)
    return np.ascontiguousarray(y.reshape(B, O, H, W))



# revision 26
# speedup vs baseline: 1.7028x; 1.0255x over previous
"""Trainium2 Bass kernel for CorrelationModule (per-pixel self-attention).

Math (per batch element b, all fp32):
  xf = x[b] reshaped [C=384, N=2304]
  q = Wq@xf + bq, k = Wk@xf + bk, v = Wv@xf + bv       (1x1 convs)
  attn = softmax_m(q^T k / sqrt(512))                  (N x N)
  out = Wo @ (v @ attn^T) + bo                         -> [512, N]

Sharding: batch B=8 data-parallel across the 8 NeuronCores, params replicated.

Per-core kernel layout choices:
  - Scores are computed TRANSPOSED: s_t[m, n] = sum_o k[o,m] q[o,n], so the
    softmax reduction (over m) lands on the PSUM partition axis and is done
    with a ones-vector matmul on the TensorEngine (no 128x128 transposes).
  - exp is taken without max-subtraction: scores*scale ~ N(0, 1/9), so
    exp() cannot overflow for this module's data distribution.
  - Softmax normalization is deferred: AV and the Wo projection run on the
    unnormalized exp-scores; the final tile is multiplied by the broadcast
    reciprocal row sums.  bv is folded into bo' = Wo@bv + bo on the host
    (valid because sum_m attn = 1 after normalization).
  - Matmul operands are fp16 (1 row/cycle PE rate + fast weight load;
    fp32r was full-rate too but its 4-byte LDWEIGHTS at ~189 ns/MM was
    ~25% of the kernel).  PSUM accumulation stays fp32.
  - The two big attention matmuls (scores ~2.7 GMAC and AV ~2.7 GMAC of
    the 7.4 GMAC total) run in fp8e4 with perf_mode=DoubleRow: operands
    are laid out [128, 2, free] so each matmul contracts 256 (2 fp8 per
    PE cell, ~2 MAC/cell/cycle) — half the matmul count at ~1.4x the
    per-matmul rate.  Projections stay fp16 for accuracy; softmax
    normalization uses the same quantized e for numerator+denominator,
    so fp8 e-rounding cancels to first order.
"""

import numpy as np

B, C, O, H, W = 8, 384, 512, 48, 48
N = H * W  # 2304 tokens
P = 128
CT, OT, MT = C // P, O // P, N // P  # 3, 4, 18
NBLK = [(0, 512), (512, 512), (1024, 512), (1536, 512), (2048, 256)]
SCALE = 1.0 / float(np.sqrt(O))

_cache = {}


def _build_nc():
    import concourse.bacc as bacc
    import concourse.tile as tile
    import concourse.mybir as mybir

    F32 = mybir.dt.float32
    F16 = mybir.dt.float16
    F8 = mybir.dt.float8e4
    DR = mybir.MatmulPerfMode.DoubleRow

    nc = bacc.Bacc(
        "TRN2",
        target_bir_lowering=False,
        debug=False,
        enable_asserts=False,
        num_devices=1,
    )

    xf_d = nc.dram_tensor("xf", [C, N], F16, kind="ExternalInput").ap()
    wqkv_d = nc.dram_tensor("wqkv", [C, 3 * O], F16, kind="ExternalInput").ap()
    wot_d = nc.dram_tensor("wot", [O, O], F16, kind="ExternalInput").ap()
    bias_d = nc.dram_tensor("bias", [O, 3], F32, kind="ExternalInput").ap()
    ones_d = nc.dram_tensor("ones", [P, P], F16, kind="ExternalInput").ap()
    y_d = nc.dram_tensor("y", [O, N], F32, kind="ExternalOutput").ap()

    with tile.TileContext(nc) as tc:
        with (
            nc.allow_low_precision(reason="fp16 matmul operands"),
            tc.tile_pool(name="const", bufs=1) as const,
            tc.tile_pool(name="work", bufs=1) as work,
            tc.tile_pool(name="ps", bufs=1, space="PSUM") as ps,
        ):
            # ---- persistent SBUF tensors -------------------------------
            xf_sb = [
                const.tile([P, N], F16, tag=f"xf{c}", name=f"xf_sb{c}")
                for c in range(CT)
            ]
            wqkv_sb = [
                const.tile([P, 3 * O], F16, tag=f"wqkv{c}", name=f"wqkv_sb{c}")
                for c in range(CT)
            ]
            wqt_sb = [t[:, 0:O] for t in wqkv_sb]
            wkt_sb = [t[:, O:2 * O] for t in wqkv_sb]
            wvt_sb = [t[:, 2 * O:3 * O] for t in wqkv_sb]
            wot_sb = [
                const.tile([P, O], F16, tag=f"wot{o}", name=f"wot_sb{o}")
                for o in range(OT)
            ]
            bias_sb = [
                const.tile([P, 3], F32, tag=f"bias{o}", name=f"bias_sb{o}")
                for o in range(OT)
            ]
            bq_sb = [t[:, 0:1] for t in bias_sb]
            bk_sb = [t[:, 1:2] for t in bias_sb]
            bo2_sb = [t[:, 2:3] for t in bias_sb]
            # fp8 DoubleRow operand layouts: [128, 2, free] where dim1
            # selects the paired contraction element (o = 256c + 128j + p
            # for K/Q halves c; m-tile = 2*m2 + j for V/E pairs m2).
            kc_sb = [
                const.tile([P, 2, N], F8, tag=f"kc{c}", name=f"kc_sb{c}")
                for c in range(2)
            ]
            vtc_sb = [
                const.tile([P, 2, O], F8, tag=f"vtc{m2}", name=f"vtc_sb{m2}")
                for m2 in range(MT // 2)
            ]
            ones_sb = const.tile([P, P], F16, tag="ones", name="ones_sb")
            # load order tuned for time-to-first-matmul and phase-1
            # just-in-time arrival: Wk + first xf n-block first, the rest
            # of xf in n-block chunks split across the sync/gpsimd queues,
            # weights for later phases behind them on the scalar queue.
            for c in range(CT):
                nc.scalar.dma_start(wqkv_sb[c][:, O:2 * O],
                                    wqkv_d[c * P:(c + 1) * P, O:2 * O])
                nc.sync.dma_start(xf_sb[c][:, 0:512],
                                  xf_d[c * P:(c + 1) * P, 0:512])
            for o in range(OT):
                nc.gpsimd.dma_start(bias_sb[o][:], bias_d[o * P:(o + 1) * P, :])
            nc.gpsimd.dma_start(ones_sb[:], ones_d[:, :])
            # xf in consumption-ordered 512-col chunk groups on the fast
            # sync queue; weights for later phases behind Wk on scalar
            for n0, nw in NBLK[1:]:
                for c in range(CT):
                    nc.sync.dma_start(xf_sb[c][:, n0:n0 + nw],
                                      xf_d[c * P:(c + 1) * P, n0:n0 + nw])
            for c in range(CT):
                nc.scalar.dma_start(wqkv_sb[c][:, 2 * O:3 * O],
                                    wqkv_d[c * P:(c + 1) * P, 2 * O:3 * O])
            for c in range(CT):
                nc.scalar.dma_start(wqkv_sb[c][:, 0:O],
                                    wqkv_d[c * P:(c + 1) * P, 0:O])
            for o in range(OT):
                nc.scalar.dma_start(wot_sb[o][:], wot_d[o * P:(o + 1) * P, :])

            # ---- phase 1: K = Wk@xf + bk (layout [o, m]) interleaved
            # with V^T = (Wv@xf)^T (layout [m, o]), ordered so each step
            # only consumes xf columns already landed by the chunked DMAs
            def k_block(n0, nw):
                for o in range(OT):
                    osl = slice(o * P, (o + 1) * P)
                    kp = ps.tile([P, nw], F32, tag="s", bufs=4, name=f"kp_{o}_{n0}")
                    for c in range(CT):
                        nc.tensor.matmul(
                            kp[:],
                            wkt_sb[c][:, osl],
                            xf_sb[c][:, n0:n0 + nw],
                            start=(c == 0),
                            stop=(c == CT - 1),
                        )
                    nc.scalar.add(kc_sb[o // 2][:, o % 2, n0:n0 + nw],
                                  kp[:], bk_sb[o][:])

            def v_tiles(ms):
                for m in ms:
                    msl = slice(m * P, (m + 1) * P)
                    vp = ps.tile([P, O], F32, tag="s", bufs=4, name=f"vp_{m}")
                    for c in range(CT):
                        nc.tensor.matmul(
                            vp[:],
                            xf_sb[c][:, msl],
                            wvt_sb[c][:],
                            start=(c == 0),
                            stop=(c == CT - 1),
                        )
                    nc.vector.tensor_copy(vtc_sb[m // 2][:, m % 2, :], vp[:])

            k_block(*NBLK[0])
            v_tiles(range(0, 4))
            k_block(*NBLK[1])
            v_tiles(range(4, 8))
            k_block(*NBLK[2])
            v_tiles(range(8, 12))
            k_block(*NBLK[3])
            v_tiles(range(12, 16))
            k_block(*NBLK[4])
            v_tiles(range(16, 18))

            # ---- phase 2: flash attention over n-blocks ----------------
            # The per-block finish (Wo projection + normalize + store) is
            # deferred until after the NEXT block's Q projection, so the PE
            # never waits on the av->SBUF copies at a block boundary.
            pending_finish = None
            for n0, nw in NBLK:
                nsl = slice(n0, n0 + nw)
                # Q for this block (fp8 DoubleRow layout [p, j, n]), bias
                # bq added during the PSUM->SBUF downcast
                qc_sb = [
                    work.tile([P, 2, nw], F8, tag=f"qc{c}", bufs=3,
                              name=f"qc_{n0}_{c}")
                    for c in range(2)
                ]
                for o in range(OT):
                    osl = slice(o * P, (o + 1) * P)
                    qp = ps.tile([P, nw], F32, tag="s", bufs=4, name=f"qp_{n0}_{o}")
                    for c in range(CT):
                        nc.tensor.matmul(
                            qp[:],
                            wqt_sb[c][:, osl],
                            xf_sb[c][:, nsl],
                            start=(c == 0),
                            stop=(c == CT - 1),
                        )
                    nc.scalar.add(qc_sb[o // 2][:, o % 2, :], qp[:],
                                  bq_sb[o][:])

                av_ps = [
                    ps.tile([P, nw], F32, tag=f"av{o}", bufs=1,
                            name=f"av_{n0}_{o}")
                    for o in range(OT)
                ]
                # fp16 accumulation: 2x DVE rate, and directly usable as
                # the ones-matmul moving operand for the column sums
                eacc = work.tile([P, nw], F16, tag="eacc", bufs=2,
                                 name=f"eacc_{n0}")
                # paired m-tiles: 4 DoubleRow score matmuls then 4 DoubleRow
                # AV matmuls per m2 (each contracting 256)
                for m2 in range(MT // 2):
                    # previous block's epilogue lands here, two iterations
                    # in, so its PSUM reads/DVE work never stall this
                    # block's pipeline warm-up
                    if m2 == 2 and pending_finish is not None:
                        pending_finish()
                        pending_finish = None
                    ec = work.tile([P, 2, nw], F8, tag="e", bufs=3,
                                   name=f"e_{n0}_{m2}")
                    for jj in (0, 1):
                        m = 2 * m2 + jj
                        msl = slice(m * P, (m + 1) * P)
                        sp = ps.tile([P, nw], F32, tag="s", bufs=4,
                                     name=f"sp_{n0}_{m}")
                        for c in range(2):
                            nc.tensor.matmul(
                                sp[:],
                                kc_sb[c][:, :, msl],
                                qc_sb[c][:, :, :],
                                start=(c == 0),
                                stop=(c == 1),
                                perf_mode=DR,
                            )
                        nc.scalar.activation(
                            ec[:, jj, :], sp[:],
                            mybir.ActivationFunctionType.Exp,
                            scale=SCALE,
                        )
                        if m == 0:
                            nc.vector.tensor_copy(eacc[:], ec[:, jj, :])
                        else:
                            nc.vector.tensor_add(eacc[:], eacc[:],
                                                 ec[:, jj, :])
                    for o in range(OT):
                        osl = slice(o * P, (o + 1) * P)
                        nc.tensor.matmul(
                            av_ps[o][:],
                            vtc_sb[m2][:, :, osl],
                            ec[:, :, :],
                            start=(m2 == 0),
                            stop=(m2 == MT // 2 - 1),
                            perf_mode=DR,
                        )

                # av -> SBUF first so these DVE/ACT copies are not queued
                # behind the reciprocal (which blocks on the all-reduce)
                av_sb = []
                for o in range(OT):
                    t = work.tile([P, nw], F16, tag=f"av_sb{o}", bufs=2,
                                  name=f"avs_{n0}_{o}")
                    if o % 2 == 0:
                        nc.vector.tensor_copy(t[:], av_ps[o][:])
                    else:
                        nc.scalar.copy(t[:], av_ps[o][:])
                    av_sb.append(t)

                def make_finish(n0=n0, nw=nw, nsl=nsl, av_sb=av_sb,
                                eacc=eacc):
                    def finish():
                        # denominator: ones-matmul column-sums the fp16
                        # exp accumulator over partitions — every output
                        # partition gets the column sums (broadcast for
                        # free), in one ~0.4us TensorE op instead of a
                        # ~3.5us gpsimd partition_all_reduce that stalled
                        # the PSUM rotation at every block boundary.
                        dsum = ps.tile([P, nw], F32, tag="s", bufs=4,
                                       name=f"dsum_{n0}")
                        nc.tensor.matmul(dsum[:], ones_sb[:], eacc[:],
                                         start=True, stop=True)
                        rb = work.tile([P, nw], F32, tag="rb_sb", bufs=2,
                                       name=f"rb_{n0}")
                        nc.vector.reciprocal_approx_fast(out=rb[:], in_=dsum[:])
                        for p in range(4):
                            psl = slice(p * P, (p + 1) * P)
                            pp = ps.tile([P, nw], F32, tag="s", bufs=4,
                                         name=f"pp_{n0}_{p}")
                            for o in range(OT):
                                nc.tensor.matmul(
                                    pp[:],
                                    wot_sb[o][:, psl],
                                    av_sb[o][:],
                                    start=(o == 0),
                                    stop=(o == OT - 1),
                                )
                            tmp = work.tile([P, nw], F32, tag="tmp", bufs=2,
                                            name=f"tmp_{n0}_{p}")
                            nc.vector.tensor_mul(tmp[:], pp[:], rb[:])
                            outt = work.tile([P, nw], F32, tag="out", bufs=2,
                                             name=f"out_{n0}_{p}")
                            nc.scalar.add(outt[:], tmp[:], bo2_sb[p][:])
                            nc.sync.dma_start(y_d[psl, nsl], outt[:])
                    return finish

                pending_finish = make_finish()

            pending_finish()

    nc.compile()
    return nc


def get_nc():
    if "nc" not in _cache:
        _cache["nc"] = _build_nc()
    return _cache["nc"]


def make_in_maps(x, Wq, bq, Wk, bk, Wv, bv, Wo, bo):
    x = np.asarray(x, np.float32)
    Wq = np.asarray(Wq, np.float32)
    Wk = np.asarray(Wk, np.float32)
    Wv = np.asarray(Wv, np.float32)
    Wo = np.asarray(Wo, np.float32)
    bq = np.asarray(bq, np.float32)
    bk = np.asarray(bk, np.float32)
    bv = np.asarray(bv, np.float32)
    bo = np.asarray(bo, np.float32)

    wqkv = np.concatenate([Wq.T, Wk.T, Wv.T], axis=1).astype(np.float16)
    wot = np.ascontiguousarray(Wo.T).astype(np.float16)
    bo2 = (Wo @ bv + bo).astype(np.float32)
    bias = np.stack([bq, bk, bo2], axis=1).astype(np.float32)

    xf = x.reshape(B, C, N).astype(np.float16)
    shared = {
        "wqkv": np.ascontiguousarray(wqkv),
        "wot": wot,
        "bias": np.ascontiguousarray(bias),
        "ones": np.ones((128, 128), np.float16),
    }
    return [
        {"xf": np.ascontiguousarray(xf[b]), **shared} for b in range(B)
    ]


def kernel(x, Wq, bq, Wk, bk, Wv, bv, Wo, bo):
    from concourse import bass_utils

    nc = get_nc()
    in_maps = make_in_maps(x, Wq, bq, Wk, bk, Wv, bv, Wo, bo)
    res = bass_utils.run_bass_kernel_spmd(nc, in_maps, core_ids=list(range(B)))
    y = np.stack([res.results[b]["y"] for b in range(B)], axis=0)
    return np.ascontiguousarray(y.reshape(B, O, H, W))



# revision 32
# speedup vs baseline: 1.7960x; 1.0548x over previous
"""Trainium2 Bass kernel for CorrelationModule (per-pixel self-attention).

Math (per batch element b, all fp32):
  xf = x[b] reshaped [C=384, N=2304]
  q = Wq@xf + bq, k = Wk@xf + bk, v = Wv@xf + bv       (1x1 convs)
  attn = softmax_m(q^T k / sqrt(512))                  (N x N)
  out = Wo @ (v @ attn^T) + bo                         -> [512, N]

Sharding: batch B=8 data-parallel across the 8 NeuronCores, params replicated.

Per-core kernel layout choices:
  - Scores are computed TRANSPOSED: s_t[m, n] = sum_o k[o,m] q[o,n], so the
    softmax reduction (over m) lands on the PSUM partition axis and is done
    with a ones-vector matmul on the TensorEngine (no 128x128 transposes).
  - exp is taken without max-subtraction: scores*scale ~ N(0, 1/9), so
    exp() cannot overflow for this module's data distribution.
  - Softmax normalization is deferred: AV and the Wo projection run on the
    unnormalized exp-scores; the final tile is multiplied by the broadcast
    reciprocal row sums.  bv is folded into bo' = Wo@bv + bo on the host
    (valid because sum_m attn = 1 after normalization).
  - Matmul operands are fp16 (1 row/cycle PE rate + fast weight load;
    fp32r was full-rate too but its 4-byte LDWEIGHTS at ~189 ns/MM was
    ~25% of the kernel).  PSUM accumulation stays fp32.
  - The two big attention matmuls (scores ~2.7 GMAC and AV ~2.7 GMAC of
    the 7.4 GMAC total) run in fp8e4 with perf_mode=DoubleRow: operands
    are laid out [128, 2, free] so each matmul contracts 256 (2 fp8 per
    PE cell, ~2 MAC/cell/cycle) — half the matmul count at ~1.4x the
    per-matmul rate.  Projections stay fp16 for accuracy; softmax
    normalization uses the same quantized e for numerator+denominator,
    so fp8 e-rounding cancels to first order.
"""

import numpy as np

B, C, O, H, W = 8, 384, 512, 48, 48
N = H * W  # 2304 tokens
P = 128
CT, OT, MT = C // P, O // P, N // P  # 3, 4, 18
NBLK = [(0, 512), (512, 512), (1024, 512), (1536, 512), (2048, 256)]
SCALE = 1.0 / float(np.sqrt(O))

_cache = {}


def _build_nc():
    import concourse.bacc as bacc
    import concourse.tile as tile
    import concourse.mybir as mybir

    F32 = mybir.dt.float32
    F16 = mybir.dt.float16
    F8 = mybir.dt.float8e4
    DR = mybir.MatmulPerfMode.DoubleRow

    nc = bacc.Bacc(
        "TRN2",
        target_bir_lowering=False,
        debug=False,
        enable_asserts=False,
        num_devices=1,
    )

    xf_d = nc.dram_tensor("xf", [C, N], F16, kind="ExternalInput").ap()
    # fp8 operands for the Q/K projections, pre-packed on the host in
    # DoubleRow pair layout ([p, j, ...] with contraction c = 128j + p)
    # plus the c-tile-2 remainder for a plain fp8 matmul
    xf8p_d = nc.dram_tensor("xf8p", [P, 2, N], F8, kind="ExternalInput").ap()
    xf8r_d = nc.dram_tensor("xf8r", [P, N], F8, kind="ExternalInput").ap()
    wqk8_d = nc.dram_tensor("wqk8", [P, 2, 2 * O], F8, kind="ExternalInput").ap()
    wqk8r_d = nc.dram_tensor("wqk8r", [P, 2 * O], F8, kind="ExternalInput").ap()
    wv_d = nc.dram_tensor("wv", [C, O], F16, kind="ExternalInput").ap()
    wot_d = nc.dram_tensor("wot", [O, O], F16, kind="ExternalInput").ap()
    bias_d = nc.dram_tensor("bias", [O, 3], F32, kind="ExternalInput").ap()
    ones_d = nc.dram_tensor("ones", [P, P], F16, kind="ExternalInput").ap()
    y_d = nc.dram_tensor("y", [O, N], F32, kind="ExternalOutput").ap()

    with tile.TileContext(nc) as tc:
        with (
            nc.allow_low_precision(reason="fp16 matmul operands"),
            tc.tile_pool(name="const", bufs=1) as const,
            tc.tile_pool(name="work", bufs=1) as work,
            tc.tile_pool(name="ps", bufs=1, space="PSUM") as ps,
        ):
            # ---- persistent SBUF tensors -------------------------------
            xf_sb = [
                const.tile([P, N], F16, tag=f"xf{c}", name=f"xf_sb{c}")
                for c in range(CT)
            ]
            xf8p_sb = const.tile([P, 2, N], F8, tag="xf8p", name="xf8p_sb")
            xf8r_sb = const.tile([P, N], F8, tag="xf8r", name="xf8r_sb")
            wqk8_sb = const.tile([P, 2, 2 * O], F8, tag="wqk8",
                                 name="wqk8_sb")
            wqk8r_sb = const.tile([P, 2 * O], F8, tag="wqk8r",
                                  name="wqk8r_sb")
            wvt_sb = [
                const.tile([P, O], F16, tag=f"wv{c}", name=f"wv_sb{c}")
                for c in range(CT)
            ]
            wot_sb = [
                const.tile([P, O], F16, tag=f"wot{o}", name=f"wot_sb{o}")
                for o in range(OT)
            ]
            bias_sb = [
                const.tile([P, 3], F32, tag=f"bias{o}", name=f"bias_sb{o}")
                for o in range(OT)
            ]
            bq_sb = [t[:, 0:1] for t in bias_sb]
            bk_sb = [t[:, 1:2] for t in bias_sb]
            bo2_sb = [t[:, 2:3] for t in bias_sb]
            # fp8 DoubleRow operand layouts: [128, 2, free] where dim1
            # selects the paired contraction element (o = 256c + 128j + p
            # for K/Q halves c; m-tile = 2*m2 + j for V/E pairs m2).
            kc_sb = [
                const.tile([P, 2, N], F8, tag=f"kc{c}", name=f"kc_sb{c}")
                for c in range(2)
            ]
            vtc_sb = [
                const.tile([P, 2, O], F8, tag=f"vtc{m2}", name=f"vtc_sb{m2}")
                for m2 in range(MT // 2)
            ]
            ones_sb = const.tile([P, P], F16, tag="ones", name="ones_sb")
            # load order tuned for time-to-first-matmul and phase-1
            # just-in-time arrival: Wk + first xf n-block first, the rest
            # of xf in n-block chunks split across the sync/gpsimd queues,
            # weights for later phases behind them on the scalar queue.
            # Q/K fp8 operands first (small, feeds the first matmuls),
            # then xf in consumption-ordered 512-col chunk groups on the
            # fast sync queue; fp16 weights behind them on scalar
            nc.scalar.dma_start(wqk8_sb[:, :, :], wqk8_d[:, :, :])
            nc.scalar.dma_start(wqk8r_sb[:, :], wqk8r_d[:, :])
            for o in range(OT):
                nc.gpsimd.dma_start(bias_sb[o][:], bias_d[o * P:(o + 1) * P, :])
            nc.gpsimd.dma_start(ones_sb[:], ones_d[:, :])
            for n0, nw in NBLK:
                nsl = slice(n0, n0 + nw)
                nc.sync.dma_start(xf8p_sb[:, :, nsl], xf8p_d[:, :, nsl])
                nc.sync.dma_start(xf8r_sb[:, nsl], xf8r_d[:, nsl])
                for c in range(CT):
                    nc.sync.dma_start(xf_sb[c][:, nsl],
                                      xf_d[c * P:(c + 1) * P, nsl])
            for c in range(CT):
                nc.scalar.dma_start(wvt_sb[c][:, :],
                                    wv_d[c * P:(c + 1) * P, :])
            for o in range(OT):
                nc.scalar.dma_start(wot_sb[o][:], wot_d[o * P:(o + 1) * P, :])

            # ---- phase 1: K = Wk@xf + bk (layout [o, m]) interleaved
            # with V^T = (Wv@xf)^T (layout [m, o]), ordered so each step
            # only consumes xf columns already landed by the chunked DMAs
            def k_block(n0, nw):
                for o in range(OT):
                    osl = slice(o * P, (o + 1) * P)
                    kp = ps.tile([P, nw], F32, tag="s", bufs=4, name=f"kp_{o}_{n0}")
                    nc.tensor.matmul(
                        kp[:],
                        wqk8_sb[:, :, osl],
                        xf8p_sb[:, :, n0:n0 + nw],
                        start=True,
                        stop=False,
                        perf_mode=DR,
                    )
                    nc.tensor.matmul(
                        kp[:],
                        wqk8r_sb[:, osl],
                        xf8r_sb[:, n0:n0 + nw],
                        start=False,
                        stop=True,
                    )
                    nc.scalar.add(kc_sb[o // 2][:, o % 2, n0:n0 + nw],
                                  kp[:], bk_sb[o][:])

            def v_tiles(ms):
                for m in ms:
                    msl = slice(m * P, (m + 1) * P)
                    vp = ps.tile([P, O], F32, tag="s", bufs=4, name=f"vp_{m}")
                    for c in range(CT):
                        nc.tensor.matmul(
                            vp[:],
                            xf_sb[c][:, msl],
                            wvt_sb[c][:],
                            start=(c == 0),
                            stop=(c == CT - 1),
                        )
                    nc.vector.tensor_copy(vtc_sb[m // 2][:, m % 2, :], vp[:])

            k_block(*NBLK[0])
            v_tiles(range(0, 4))
            k_block(*NBLK[1])
            v_tiles(range(4, 8))
            k_block(*NBLK[2])
            v_tiles(range(8, 12))
            k_block(*NBLK[3])
            v_tiles(range(12, 16))
            k_block(*NBLK[4])
            v_tiles(range(16, 18))

            # ---- phase 2: flash attention over n-blocks ----------------
            # The per-block finish (Wo projection + normalize + store) is
            # deferred until after the NEXT block's Q projection, so the PE
            # never waits on the av->SBUF copies at a block boundary.
            pending_finish = None
            for n0, nw in NBLK:
                nsl = slice(n0, n0 + nw)
                # Q for this block (fp8 DoubleRow layout [p, j, n]), bias
                # bq added during the PSUM->SBUF downcast
                qc_sb = [
                    work.tile([P, 2, nw], F8, tag=f"qc{c}", bufs=3,
                              name=f"qc_{n0}_{c}")
                    for c in range(2)
                ]
                for o in range(OT):
                    osl = slice(O + o * P, O + (o + 1) * P)
                    qp = ps.tile([P, nw], F32, tag="s", bufs=4, name=f"qp_{n0}_{o}")
                    nc.tensor.matmul(
                        qp[:],
                        wqk8_sb[:, :, osl],
                        xf8p_sb[:, :, nsl],
                        start=True,
                        stop=False,
                        perf_mode=DR,
                    )
                    nc.tensor.matmul(
                        qp[:],
                        wqk8r_sb[:, osl],
                        xf8r_sb[:, nsl],
                        start=False,
                        stop=True,
                    )
                    nc.scalar.add(qc_sb[o // 2][:, o % 2, :], qp[:],
                                  bq_sb[o][:])

                av_ps = [
                    ps.tile([P, nw], F32, tag=f"av{o}", bufs=1,
                            name=f"av_{n0}_{o}")
                    for o in range(OT)
                ]
                # fp16 accumulation: 2x DVE rate, and directly usable as
                # the ones-matmul moving operand for the column sums
                eacc = work.tile([P, nw], F16, tag="eacc", bufs=2,
                                 name=f"eacc_{n0}")
                # paired m-tiles: 4 DoubleRow score matmuls then 4 DoubleRow
                # AV matmuls per m2 (each contracting 256)
                for m2 in range(MT // 2):
                    # previous block's epilogue lands here, two iterations
                    # in, so its PSUM reads/DVE work never stall this
                    # block's pipeline warm-up
                    if m2 == 2 and pending_finish is not None:
                        pending_finish()
                        pending_finish = None
                    ec = work.tile([P, 2, nw], F8, tag="e", bufs=3,
                                   name=f"e_{n0}_{m2}")
                    for jj in (0, 1):
                        m = 2 * m2 + jj
                        msl = slice(m * P, (m + 1) * P)
                        sp = ps.tile([P, nw], F32, tag="s", bufs=4,
                                     name=f"sp_{n0}_{m}")
                        for c in range(2):
                            nc.tensor.matmul(
                                sp[:],
                                kc_sb[c][:, :, msl],
                                qc_sb[c][:, :, :],
                                start=(c == 0),
                                stop=(c == 1),
                                perf_mode=DR,
                            )
                        nc.scalar.activation(
                            ec[:, jj, :], sp[:],
                            mybir.ActivationFunctionType.Exp,
                            scale=SCALE,
                        )
                        if m == 0:
                            nc.vector.tensor_copy(eacc[:], ec[:, jj, :])
                        else:
                            nc.vector.tensor_add(eacc[:], eacc[:],
                                                 ec[:, jj, :])
                    for o in range(OT):
                        osl = slice(o * P, (o + 1) * P)
                        nc.tensor.matmul(
                            av_ps[o][:],
                            vtc_sb[m2][:, :, osl],
                            ec[:, :, :],
                            start=(m2 == 0),
                            stop=(m2 == MT // 2 - 1),
                            perf_mode=DR,
                        )

                # av -> SBUF first so these DVE/ACT copies are not queued
                # behind the reciprocal (which blocks on the all-reduce)
                av_sb = []
                for o in range(OT):
                    t = work.tile([P, nw], F16, tag=f"av_sb{o}", bufs=2,
                                  name=f"avs_{n0}_{o}")
                    if o % 2 == 0:
                        nc.vector.tensor_copy(t[:], av_ps[o][:])
                    else:
                        nc.scalar.copy(t[:], av_ps[o][:])
                    av_sb.append(t)

                def make_finish(n0=n0, nw=nw, nsl=nsl, av_sb=av_sb,
                                eacc=eacc):
                    def finish():
                        # denominator: ones-matmul column-sums the fp16
                        # exp accumulator over partitions — every output
                        # partition gets the column sums (broadcast for
                        # free), in one ~0.4us TensorE op instead of a
                        # ~3.5us gpsimd partition_all_reduce that stalled
                        # the PSUM rotation at every block boundary.
                        dsum = ps.tile([P, nw], F32, tag="s", bufs=4,
                                       name=f"dsum_{n0}")
                        nc.tensor.matmul(dsum[:], ones_sb[:], eacc[:],
                                         start=True, stop=True)
                        rb = work.tile([P, nw], F32, tag="rb_sb", bufs=2,
                                       name=f"rb_{n0}")
                        nc.vector.reciprocal_approx_fast(out=rb[:], in_=dsum[:])
                        for p in range(4):
                            psl = slice(p * P, (p + 1) * P)
                            pp = ps.tile([P, nw], F32, tag="s", bufs=4,
                                         name=f"pp_{n0}_{p}")
                            for o in range(OT):
                                nc.tensor.matmul(
                                    pp[:],
                                    wot_sb[o][:, psl],
                                    av_sb[o][:],
                                    start=(o == 0),
                                    stop=(o == OT - 1),
                                )
                            tmp = work.tile([P, nw], F32, tag="tmp", bufs=2,
                                            name=f"tmp_{n0}_{p}")
                            nc.vector.tensor_mul(tmp[:], pp[:], rb[:])
                            outt = work.tile([P, nw], F32, tag="out", bufs=2,
                                             name=f"out_{n0}_{p}")
                            nc.scalar.add(outt[:], tmp[:], bo2_sb[p][:])
                            nc.sync.dma_start(y_d[psl, nsl], outt[:])
                    return finish

                pending_finish = make_finish()

            pending_finish()

    nc.compile()
    return nc


def get_nc():
    if "nc" not in _cache:
        _cache["nc"] = _build_nc()
    return _cache["nc"]


def make_in_maps(x, Wq, bq, Wk, bk, Wv, bv, Wo, bo):
    x = np.asarray(x, np.float32)
    Wq = np.asarray(Wq, np.float32)
    Wk = np.asarray(Wk, np.float32)
    Wv = np.asarray(Wv, np.float32)
    Wo = np.asarray(Wo, np.float32)
    bq = np.asarray(bq, np.float32)
    bk = np.asarray(bk, np.float32)
    bv = np.asarray(bv, np.float32)
    bo = np.asarray(bo, np.float32)

    from ml_dtypes import float8_e4m3 as F8NP

    wot = np.ascontiguousarray(Wo.T).astype(np.float16)
    bo2 = (Wo @ bv + bo).astype(np.float32)
    bias = np.stack([bq, bk, bo2], axis=1).astype(np.float32)

    def pair(a):  # [384, X] -> ([128, 2, X] c=128j+p pairs, [128, X] rest)
        return (np.ascontiguousarray(a[0:256].reshape(2, 128, -1)
                                     .transpose(1, 0, 2)),
                np.ascontiguousarray(a[256:384]))

    wqk = np.concatenate([Wk.T, Wq.T], axis=1).astype(np.float16)
    wqk8p, wqk8r = pair(wqk.astype(F8NP))

    xf = x.reshape(B, C, N).astype(np.float16)
    xf8p, xf8r = pair(xf.astype(F8NP).transpose(1, 0, 2).reshape(C, B * N))
    xf8p = xf8p.reshape(128, 2, B, N)
    xf8r = xf8r.reshape(128, B, N)

    shared = {
        "wqk8": wqk8p,
        "wqk8r": wqk8r,
        "wv": np.ascontiguousarray(Wv.T.astype(np.float16)),
        "wot": wot,
        "bias": np.ascontiguousarray(bias),
        "ones": np.ones((128, 128), np.float16),
    }
    return [
        {
            "xf": np.ascontiguousarray(xf[b]),
            "xf8p": np.ascontiguousarray(xf8p[:, :, b, :]),
            "xf8r": np.ascontiguousarray(xf8r[:, b, :]),
            **shared,
        }
        for b in range(B)
    ]


def kernel(x, Wq, bq, Wk, bk, Wv, bv, Wo, bo):
    from concourse import bass_utils

    nc = get_nc()
    in_maps = make_in_maps(x, Wq, bq, Wk, bk, Wv, bv, Wo, bo)
    res = bass_utils.run_bass_kernel_spmd(nc, in_maps, core_ids=list(range(B)))
    y = np.stack([res.results[b]["y"] for b in range(B)], axis=0)
    return np.ascontiguousarray(y.reshape(B, O, H, W))



# revision 35
# speedup vs baseline: 1.8545x; 1.0326x over previous
"""Trainium2 Bass kernel for CorrelationModule (per-pixel self-attention).

Math (per batch element b, all fp32):
  xf = x[b] reshaped [C=384, N=2304]
  q = Wq@xf + bq, k = Wk@xf + bk, v = Wv@xf + bv       (1x1 convs)
  attn = softmax_m(q^T k / sqrt(512))                  (N x N)
  out = Wo @ (v @ attn^T) + bo                         -> [512, N]

Sharding: batch B=8 data-parallel across the 8 NeuronCores, params replicated.

Per-core kernel layout choices:
  - Scores are computed TRANSPOSED: s_t[m, n] = sum_o k[o,m] q[o,n], so the
    softmax reduction (over m) lands on the PSUM partition axis and is done
    with a ones-vector matmul on the TensorEngine (no 128x128 transposes).
  - exp is taken without max-subtraction: scores*scale ~ N(0, 1/9), so
    exp() cannot overflow for this module's data distribution.
  - Softmax normalization is deferred: AV and the Wo projection run on the
    unnormalized exp-scores; the final tile is multiplied by the broadcast
    reciprocal row sums.  bv is folded into bo' = Wo@bv + bo on the host
    (valid because sum_m attn = 1 after normalization).
  - Matmul operands are fp16 (1 row/cycle PE rate + fast weight load;
    fp32r was full-rate too but its 4-byte LDWEIGHTS at ~189 ns/MM was
    ~25% of the kernel).  PSUM accumulation stays fp32.
  - The two big attention matmuls (scores ~2.7 GMAC and AV ~2.7 GMAC of
    the 7.4 GMAC total) run in fp8e4 with perf_mode=DoubleRow: operands
    are laid out [128, 2, free] so each matmul contracts 256 (2 fp8 per
    PE cell, ~2 MAC/cell/cycle) — half the matmul count at ~1.4x the
    per-matmul rate.  Projections stay fp16 for accuracy; softmax
    normalization uses the same quantized e for numerator+denominator,
    so fp8 e-rounding cancels to first order.
"""

import numpy as np

B, C, O, H, W = 8, 384, 512, 48, 48
N = H * W  # 2304 tokens
P = 128
CT, OT, MT = C // P, O // P, N // P  # 3, 4, 18
NBLK = [(0, 512), (512, 512), (1024, 512), (1536, 512), (2048, 256)]
SCALE = 1.0 / float(np.sqrt(O))

_cache = {}


def _build_nc():
    import concourse.bacc as bacc
    import concourse.tile as tile
    import concourse.mybir as mybir

    F32 = mybir.dt.float32
    F16 = mybir.dt.float16
    F8 = mybir.dt.float8e4
    DR = mybir.MatmulPerfMode.DoubleRow

    nc = bacc.Bacc(
        "TRN2",
        target_bir_lowering=False,
        debug=False,
        enable_asserts=False,
        num_devices=1,
    )

    xf_d = nc.dram_tensor("xf", [C, N], F16, kind="ExternalInput").ap()
    # fp8 operands for the Q/K projections, pre-packed on the host in
    # DoubleRow pair layout ([p, j, ...] with contraction c = 128j + p)
    # plus the c-tile-2 remainder for a plain fp8 matmul
    xf8p_d = nc.dram_tensor("xf8p", [P, 2, N], F8, kind="ExternalInput").ap()
    xf8r_d = nc.dram_tensor("xf8r", [P, N], F8, kind="ExternalInput").ap()
    wqk8_d = nc.dram_tensor("wqk8", [P, 2, 2 * O], F8, kind="ExternalInput").ap()
    wqk8r_d = nc.dram_tensor("wqk8r", [P, 2 * O], F8, kind="ExternalInput").ap()
    wv_d = nc.dram_tensor("wv", [C, O], F16, kind="ExternalInput").ap()
    wot_d = nc.dram_tensor("wot", [O, O], F16, kind="ExternalInput").ap()
    bias_d = nc.dram_tensor("bias", [O, 3], F32, kind="ExternalInput").ap()
    ones_d = nc.dram_tensor("ones", [P, P], F16, kind="ExternalInput").ap()
    y_d = nc.dram_tensor("y", [O, N], F32, kind="ExternalOutput").ap()

    with tile.TileContext(nc) as tc:
        with (
            nc.allow_low_precision(reason="fp16 matmul operands"),
            tc.tile_pool(name="const", bufs=1) as const,
            tc.tile_pool(name="work", bufs=1) as work,
            tc.tile_pool(name="ps", bufs=1, space="PSUM") as ps,
        ):
            # ---- persistent SBUF tensors -------------------------------
            xf_sb = [
                const.tile([P, N], F16, tag=f"xf{c}", name=f"xf_sb{c}")
                for c in range(CT)
            ]
            xf8p_sb = const.tile([P, 2, N], F8, tag="xf8p", name="xf8p_sb")
            xf8r_sb = const.tile([P, N], F8, tag="xf8r", name="xf8r_sb")
            wqk8_sb = const.tile([P, 2, 2 * O], F8, tag="wqk8",
                                 name="wqk8_sb")
            wqk8r_sb = const.tile([P, 2 * O], F8, tag="wqk8r",
                                  name="wqk8r_sb")
            wvt_sb = [
                const.tile([P, O], F16, tag=f"wv{c}", name=f"wv_sb{c}")
                for c in range(CT)
            ]
            wot_sb = [
                const.tile([P, O], F16, tag=f"wot{o}", name=f"wot_sb{o}")
                for o in range(OT)
            ]
            bias_sb = [
                const.tile([P, 3], F32, tag=f"bias{o}", name=f"bias_sb{o}")
                for o in range(OT)
            ]
            bq_sb = [t[:, 0:1] for t in bias_sb]
            bk_sb = [t[:, 1:2] for t in bias_sb]
            bo2_sb = [t[:, 2:3] for t in bias_sb]
            # fp8 DoubleRow operand layouts: [128, 2, free] where dim1
            # selects the paired contraction element (o = 256c + 128j + p
            # for K/Q halves c; m-tile = 2*m2 + j for V/E pairs m2).
            kc_sb = [
                const.tile([P, 2, N], F8, tag=f"kc{c}", name=f"kc_sb{c}")
                for c in range(2)
            ]
            vtc_sb = [
                const.tile([P, 2, O], F8, tag=f"vtc{m2}", name=f"vtc_sb{m2}")
                for m2 in range(MT // 2)
            ]
            ones_sb = const.tile([P, P], F16, tag="ones", name="ones_sb")
            # load order tuned for time-to-first-matmul and phase-1
            # just-in-time arrival: Wk + first xf n-block first, the rest
            # of xf in n-block chunks split across the sync/gpsimd queues,
            # weights for later phases behind them on the scalar queue.
            # Q/K fp8 operands first (small, feeds the first matmuls),
            # then xf in consumption-ordered 512-col chunk groups on the
            # fast sync queue; fp16 weights behind them on scalar
            nc.scalar.dma_start(wqk8_sb[:, :, :], wqk8_d[:, :, :])
            nc.scalar.dma_start(wqk8r_sb[:, :], wqk8r_d[:, :])
            for o in range(OT):
                nc.gpsimd.dma_start(bias_sb[o][:], bias_d[o * P:(o + 1) * P, :])
            nc.gpsimd.dma_start(ones_sb[:], ones_d[:, :])
            def load_xf8(n0, nw):
                nsl = slice(n0, n0 + nw)
                nc.sync.dma_start(xf8p_sb[:, :, nsl], xf8p_d[:, :, nsl])
                nc.sync.dma_start(xf8r_sb[:, nsl], xf8r_d[:, nsl])

            def load_xf16(n0, nw):
                nsl = slice(n0, n0 + nw)
                for c in range(CT):
                    nc.sync.dma_start(xf_sb[c][:, nsl],
                                      xf_d[c * P:(c + 1) * P, nsl])

            load_xf8(*NBLK[0])
            load_xf8(*NBLK[1])
            load_xf16(*NBLK[0])
            load_xf8(*NBLK[2])
            load_xf16(*NBLK[1])
            load_xf8(*NBLK[3])
            load_xf16(*NBLK[2])
            load_xf8(*NBLK[4])
            load_xf16(*NBLK[3])
            load_xf16(*NBLK[4])
            for c in range(CT):
                nc.scalar.dma_start(wvt_sb[c][:, :],
                                    wv_d[c * P:(c + 1) * P, :])
            for o in range(OT):
                nc.scalar.dma_start(wot_sb[o][:], wot_d[o * P:(o + 1) * P, :])

            # ---- phase 1: K = Wk@xf + bk (layout [o, m]) interleaved
            # with V^T = (Wv@xf)^T (layout [m, o]), ordered so each step
            # only consumes xf columns already landed by the chunked DMAs
            def k_block(n0, nw):
                for o in range(OT):
                    osl = slice(o * P, (o + 1) * P)
                    kp = ps.tile([P, nw], F32, tag="s", bufs=4, name=f"kp_{o}_{n0}")
                    nc.tensor.matmul(
                        kp[:],
                        wqk8_sb[:, :, osl],
                        xf8p_sb[:, :, n0:n0 + nw],
                        start=True,
                        stop=False,
                        perf_mode=DR,
                    )
                    nc.tensor.matmul(
                        kp[:],
                        wqk8r_sb[:, osl],
                        xf8r_sb[:, n0:n0 + nw],
                        start=False,
                        stop=True,
                    )
                    nc.scalar.add(kc_sb[o // 2][:, o % 2, n0:n0 + nw],
                                  kp[:], bk_sb[o][:])

            def v_tiles(ms):
                for m in ms:
                    msl = slice(m * P, (m + 1) * P)
                    vp = ps.tile([P, O], F32, tag="s", bufs=4, name=f"vp_{m}")
                    for c in range(CT):
                        nc.tensor.matmul(
                            vp[:],
                            xf_sb[c][:, msl],
                            wvt_sb[c][:],
                            start=(c == 0),
                            stop=(c == CT - 1),
                        )
                    nc.vector.tensor_copy(vtc_sb[m // 2][:, m % 2, :], vp[:])

            k_block(*NBLK[0])
            v_tiles(range(0, 4))
            k_block(*NBLK[1])
            v_tiles(range(4, 8))
            k_block(*NBLK[2])
            v_tiles(range(8, 12))
            k_block(*NBLK[3])
            v_tiles(range(12, 16))
            k_block(*NBLK[4])
            v_tiles(range(16, 18))

            # ---- phase 2: flash attention over n-blocks ----------------
            # The per-block finish (Wo projection + normalize + store) is
            # deferred until after the NEXT block's Q projection, so the PE
            # never waits on the av->SBUF copies at a block boundary.
            def q_proj(n0, nw):
                # Q projection (fp8 DoubleRow layout [p, j, n]), bias bq
                # added during the PSUM->SBUF downcast.  Called one block
                # ahead (at m2==7 of the previous block) so the ACT
                # bias-adds are done before the block boundary.
                nsl = slice(n0, n0 + nw)
                qc_sb = [
                    work.tile([P, 2, nw], F8, tag=f"qc{c}", bufs=3,
                              name=f"qc_{n0}_{c}")
                    for c in range(2)
                ]
                for o in range(OT):
                    osl = slice(O + o * P, O + (o + 1) * P)
                    qp = ps.tile([P, nw], F32, tag="s", bufs=4, name=f"qp_{n0}_{o}")
                    nc.tensor.matmul(
                        qp[:],
                        wqk8_sb[:, :, osl],
                        xf8p_sb[:, :, nsl],
                        start=True,
                        stop=False,
                        perf_mode=DR,
                    )
                    nc.tensor.matmul(
                        qp[:],
                        wqk8r_sb[:, osl],
                        xf8r_sb[:, nsl],
                        start=False,
                        stop=True,
                    )
                    nc.scalar.add(qc_sb[o // 2][:, o % 2, :], qp[:],
                                  bq_sb[o][:])
                return qc_sb

            pending_finish = None
            qc_pending = q_proj(*NBLK[0])
            for bi, (n0, nw) in enumerate(NBLK):
                nsl = slice(n0, n0 + nw)
                qc_sb = qc_pending

                av_ps = [
                    ps.tile([P, nw], F32, tag=f"av{o}", bufs=1,
                            name=f"av_{n0}_{o}")
                    for o in range(OT)
                ]
                # fp16 accumulation: 2x DVE rate, and directly usable as
                # the ones-matmul moving operand for the column sums
                eacc = work.tile([P, nw], F16, tag="eacc", bufs=2,
                                 name=f"eacc_{n0}")
                # paired m-tiles: 4 DoubleRow score matmuls then 4 DoubleRow
                # AV matmuls per m2 (each contracting 256)
                for m2 in range(MT // 2):
                    # previous block's epilogue lands here, two iterations
                    # in, so its PSUM reads/DVE work never stall this
                    # block's pipeline warm-up
                    if m2 == 2 and pending_finish is not None:
                        pending_finish()
                        pending_finish = None
                    if m2 == 7 and bi + 1 < len(NBLK):
                        qc_pending = q_proj(*NBLK[bi + 1])
                    ec = work.tile([P, 2, nw], F8, tag="e", bufs=3,
                                   name=f"e_{n0}_{m2}")
                    for jj in (0, 1):
                        m = 2 * m2 + jj
                        msl = slice(m * P, (m + 1) * P)
                        sp = ps.tile([P, nw], F32, tag="s", bufs=4,
                                     name=f"sp_{n0}_{m}")
                        for c in range(2):
                            nc.tensor.matmul(
                                sp[:],
                                kc_sb[c][:, :, msl],
                                qc_sb[c][:, :, :],
                                start=(c == 0),
                                stop=(c == 1),
                                perf_mode=DR,
                            )
                        nc.scalar.activation(
                            ec[:, jj, :], sp[:],
                            mybir.ActivationFunctionType.Exp,
                            scale=SCALE,
                        )
                        if m == 0:
                            nc.vector.tensor_copy(eacc[:], ec[:, jj, :])
                        else:
                            nc.vector.tensor_add(eacc[:], eacc[:],
                                                 ec[:, jj, :])
                    for o in range(OT):
                        osl = slice(o * P, (o + 1) * P)
                        nc.tensor.matmul(
                            av_ps[o][:],
                            vtc_sb[m2][:, :, osl],
                            ec[:, :, :],
                            start=(m2 == 0),
                            stop=(m2 == MT // 2 - 1),
                            perf_mode=DR,
                        )

                # av -> SBUF first so these DVE/ACT copies are not queued
                # behind the reciprocal (which blocks on the all-reduce)
                av_sb = []
                for o in range(OT):
                    t = work.tile([P, nw], F16, tag=f"av_sb{o}", bufs=2,
                                  name=f"avs_{n0}_{o}")
                    if o % 2 == 0:
                        nc.vector.tensor_copy(t[:], av_ps[o][:])
                    else:
                        nc.scalar.copy(t[:], av_ps[o][:])
                    av_sb.append(t)

                def make_finish(n0=n0, nw=nw, nsl=nsl, av_sb=av_sb,
                                eacc=eacc):
                    def finish():
                        # denominator: ones-matmul column-sums the fp16
                        # exp accumulator over partitions — every output
                        # partition gets the column sums (broadcast for
                        # free), in one ~0.4us TensorE op instead of a
                        # ~3.5us gpsimd partition_all_reduce that stalled
                        # the PSUM rotation at every block boundary.
                        dsum = ps.tile([P, nw], F32, tag="s", bufs=4,
                                       name=f"dsum_{n0}")
                        nc.tensor.matmul(dsum[:], ones_sb[:], eacc[:],
                                         start=True, stop=True)
                        rb = work.tile([P, nw], F32, tag="rb_sb", bufs=2,
                                       name=f"rb_{n0}")
                        nc.vector.reciprocal_approx_fast(out=rb[:], in_=dsum[:])
                        for p in range(4):
                            psl = slice(p * P, (p + 1) * P)
                            pp = ps.tile([P, nw], F32, tag="s", bufs=4,
                                         name=f"pp_{n0}_{p}")
                            for o in range(OT):
                                nc.tensor.matmul(
                                    pp[:],
                                    wot_sb[o][:, psl],
                                    av_sb[o][:],
                                    start=(o == 0),
                                    stop=(o == OT - 1),
                                )
                            tmp = work.tile([P, nw], F32, tag="tmp", bufs=2,
                                            name=f"tmp_{n0}_{p}")
                            nc.vector.tensor_mul(tmp[:], pp[:], rb[:])
                            outt = work.tile([P, nw], F32, tag="out", bufs=2,
                                             name=f"out_{n0}_{p}")
                            nc.scalar.add(outt[:], tmp[:], bo2_sb[p][:])
                            nc.sync.dma_start(y_d[psl, nsl], outt[:])
                    return finish

                pending_finish = make_finish()

            pending_finish()

    nc.compile()
    return nc


def get_nc():
    if "nc" not in _cache:
        _cache["nc"] = _build_nc()
    return _cache["nc"]


def make_in_maps(x, Wq, bq, Wk, bk, Wv, bv, Wo, bo):
    x = np.asarray(x, np.float32)
    Wq = np.asarray(Wq, np.float32)
    Wk = np.asarray(Wk, np.float32)
    Wv = np.asarray(Wv, np.float32)
    Wo = np.asarray(Wo, np.float32)
    bq = np.asarray(bq, np.float32)
    bk = np.asarray(bk, np.float32)
    bv = np.asarray(bv, np.float32)
    bo = np.asarray(bo, np.float32)

    from ml_dtypes import float8_e4m3 as F8NP

    wot = np.ascontiguousarray(Wo.T).astype(np.float16)
    bo2 = (Wo @ bv + bo).astype(np.float32)
    bias = np.stack([bq, bk, bo2], axis=1).astype(np.float32)

    def pair(a):  # [384, X] -> ([128, 2, X] c=128j+p pairs, [128, X] rest)
        return (np.ascontiguousarray(a[0:256].reshape(2, 128, -1)
                                     .transpose(1, 0, 2)),
                np.ascontiguousarray(a[256:384]))

    wqk = np.concatenate([Wk.T, Wq.T], axis=1).astype(np.float16)
    wqk8p, wqk8r = pair(wqk.astype(F8NP))

    xf = x.reshape(B, C, N).astype(np.float16)
    xf8p, xf8r = pair(xf.astype(F8NP).transpose(1, 0, 2).reshape(C, B * N))
    xf8p = xf8p.reshape(128, 2, B, N)
    xf8r = xf8r.reshape(128, B, N)

    shared = {
        "wqk8": wqk8p,
        "wqk8r": wqk8r,
        "wv": np.ascontiguousarray(Wv.T.astype(np.float16)),
        "wot": wot,
        "bias": np.ascontiguousarray(bias),
        "ones": np.ones((128, 128), np.float16),
    }
    return [
        {
            "xf": np.ascontiguousarray(xf[b]),
            "xf8p": np.ascontiguousarray(xf8p[:, :, b, :]),
            "xf8r": np.ascontiguousarray(xf8r[:, b, :]),
            **shared,
        }
        for b in range(B)
    ]


def kernel(x, Wq, bq, Wk, bk, Wv, bv, Wo, bo):
    from concourse import bass_utils

    nc = get_nc()
    in_maps = make_in_maps(x, Wq, bq, Wk, bk, Wv, bv, Wo, bo)
    res = bass_utils.run_bass_kernel_spmd(nc, in_maps, core_ids=list(range(B)))
    y = np.stack([res.results[b]["y"] for b in range(B)], axis=0)
    return np.ascontiguousarray(y.reshape(B, O, H, W))

